# revision 1
# baseline (speedup 1.0000x reference)
"""GCN-5 message-passing kernel for Trainium2, 8-core SPMD Bass/Tile.

Strategy (graph-parallel, per the sharding hint):
  - batch is sorted, so graphs are contiguous node ranges.  Core c owns graphs
    [125c, 125(c+1)) and their nodes; edges are assigned to the core owning the
    dst node.  Pooling / layernorm / output head are fully local per core.
  - Per layer each core computes z = h @ W for its own node slice on the
    TensorEngine, the slices are AllGathered (device collective), and each core
    gathers z[src] rows for its edges with indirect DMA.
  - Scatter-add per 128-node tile is a one-hot matmul on the TensorEngine with
    PSUM accumulation (edges pre-sorted by dst tile and padded to B blocks of
    128 on the host; padded edges carry norm=0 so they contribute nothing).
  - The symmetric normalization deg^-1/2 and all index/one-hot inputs are
    precomputed on the host from edge_index/batch.
"""
import sys
import types
import contextlib

import numpy as np

sys.path.insert(0, "/opt/trn_rl_repo")

import concourse.bass as bass
import concourse.tile as tile
from concourse import mybir
from concourse.masks import make_identity
from concourse.vector_clock import ScopedClock

F32 = mybir.dt.float32
I32 = mybir.dt.int32
M = 8  # NeuronCores
H = 64

# ---------------------------------------------------------------------------
# Environment fixes for this container
# ---------------------------------------------------------------------------

def _install_env_fixes():
    import concourse.tile as tile_mod

    def _patched_drain_and_barrier(self, tick_clock, wait_clock):
        # this walrus build allows a single sync-wait per TPB_CTRL Drain;
        # split the Tile tail-drain's waits across multiple drains.
        nc = self.nc
        drain_inst = nc.sync.drain()
        wait_clock.add_sem_waits(drain_inst.ins,
                                 ScopedClock({None: tick_clock.global_clock}))
        si = drain_inst.ins.sync_info
        waits = list(si.on_wait or [])
        if len(waits) > 1:
            si.on_wait[:] = waits[:1]
            for w in waits[1:]:
                d2 = nc.sync.drain()
                if d2.ins.sync_info is None:
                    d2.ins.sync_info = mybir.SyncInfo(on_wait=[w], on_update=[])
                else:
                    d2.ins.sync_info.on_wait.append(w)
        nc.all_engine_barrier()
        assert self.sems is not None
        popped = nc._tile_sem_poison_stack.pop()
        assert popped is self._sem_poison
        nc.clear_and_free_semaphores(list(self.sems.allocated().values()))
        nc.all_engine_barrier()

    tile_mod.TileContext._drain_and_barrier = _patched_drain_and_barrier

    _orig_lower_ordered = tile_mod.TileContext._lower_ordered_insts

    def _split_multiwait_lower(self, ordered):
        nc = self.nc
        for bbname, insts in ordered.items():
            newlist = []
            changed = False
            for inst in insts:
                si = getattr(inst, "sync_info", None)
                eng = getattr(inst, "engine", None)
                if (si is not None and si.on_wait and len(si.on_wait) > 1
                        and eng is not None and eng != mybir.EngineType.Unassigned
                        and inst.is_executable()):
                    waits = list(si.on_wait)
                    si.on_wait[:] = waits[-1:]
                    for w in waits[:-1]:
                        nop = mybir.InstNoOp(
                            name=nc.get_next_instruction_name(), engine=eng)
                        nop.sync_info = mybir.SyncInfo(on_wait=[w], on_update=[])
                        try:
                            nc.register_instruction(nop, overwrite=True)
                        except Exception:
                            pass
                        newlist.append(nop)
                    changed = True
                newlist.append(inst)
            if changed:
                insts[:] = newlist
        return _orig_lower_ordered(self, ordered)

    if getattr(tile_mod.TileContext._lower_ordered_insts, "__name__", "") != \
            "_split_multiwait_lower":
        tile_mod.TileContext._lower_ordered_insts = _split_multiwait_lower

    # NTFF profile hook (lets trace=True work under axon); best-effort.
    if "antenv.axon_hooks" not in sys.modules:
        try:
            from trn_agent_boot.trn_boot import _ntff_profile_via_ctypes
            hook = _ntff_profile_via_ctypes("/opt/axon/libaxon_pjrt.so")
            mod = types.ModuleType("antenv.axon_hooks")
            mod.get_axon_ntff_profile_hook = lambda: hook
            mod.set_axon_ntff_profile_hook = lambda h: None
            sys.modules["antenv.axon_hooks"] = mod
            import antenv
            antenv.axon_hooks = mod
        except Exception:
            pass


_install_env_fixes()

# ---------------------------------------------------------------------------
# Host preprocessing
# ---------------------------------------------------------------------------

def preprocess(x, edge_index, batch, n_graphs):
    N = x.shape[0]
    GPC = n_graphs // M
    src = np.concatenate([edge_index[0], np.arange(N, dtype=np.int64)])
    dst = np.concatenate([edge_index[1], np.arange(N, dtype=np.int64)])
    deg = np.bincount(dst, minlength=N).astype(np.float32)
    dis = 1.0 / np.sqrt(deg)
    norm = (dis[src] * dis[dst]).astype(np.float32)

    batch = np.asarray(batch)
    owner = (batch // GPC).astype(np.int64)
    node_start = np.searchsorted(batch, np.arange(M) * GPC)
    node_end = np.searchsorted(batch, np.arange(M) * GPC + GPC)
    n_c = node_end - node_start
    T = int(np.ceil(n_c.max() / 128))
    S = T * 128
    local_pos = np.arange(N) - node_start[owner]
    padded_idx = (owner * S + local_pos).astype(np.int32)

    eo = owner[dst]
    ld_all = local_pos[dst]

    counts_ct = np.zeros((M, T), np.int64)
    for c in range(M):
        sel = eo == c
        counts_ct[c] = np.bincount(ld_all[sel] // 128, minlength=T)
    B = int(np.ceil(counts_ct.max() / 128))

    cnt_g = np.bincount(batch, minlength=n_graphs).astype(np.float32)

    in_maps = []
    for c in range(M):
        sel = np.nonzero(eo == c)[0]
        es, en, ld = src[sel], norm[sel], ld_all[sel]
        tid = ld // 128
        order = np.argsort(tid, kind="stable")
        es, en, ld, tid = es[order], en[order], ld[order], tid[order]
        counts = counts_ct[c]
        cum = np.concatenate([[0], np.cumsum(counts)])
        pos = np.arange(len(es)) - np.repeat(cum[:-1], counts)
        dest = tid * (B * 128) + pos

        gsrc = np.zeros(T * B * 128, np.int32)
        nrm = np.zeros(T * B * 128, np.float32)
        ldst = np.zeros(T * B * 128, np.float32)
        gsrc[dest] = padded_idx[es]
        nrm[dest] = en
        ldst[dest] = ld % 128

        def to_sb(a):
            return np.ascontiguousarray(
                a.reshape(T, B, 128).transpose(2, 0, 1).reshape(128, T * B))

        xT = np.zeros((128, S), np.float32)
        xs = x[node_start[c]:node_end[c]]
        xT[:, :n_c[c]] = xs.T

        gflat = np.full(T * 128, -1.0, np.float32)
        gflat[:n_c[c]] = batch[node_start[c]:node_end[c]] - c * GPC
        gcol = gflat.reshape(T, 128).T

        invcnt = np.zeros((128, 1), np.float32)
        invcnt[:GPC, 0] = 1.0 / np.maximum(cnt_g[c * GPC:(c + 1) * GPC], 1.0)

        iota = np.broadcast_to(np.arange(128, dtype=np.float32), (128, 128)).copy()

        in_maps.append(dict(
            xT=xT, idx=to_sb(gsrc), nrm=to_sb(nrm), ldst=to_sb(ldst),
            gcol=np.ascontiguousarray(gcol), invcnt=invcnt, iota=iota,
        ))
    meta = dict(T=T, B=B, S=S, GPC=GPC)
    return in_maps, meta


def make_weight_inputs(W1, b1, Wh, bh, Wout, bout):
    return dict(
        W1=np.ascontiguousarray(W1, np.float32),
        Wh=np.ascontiguousarray(Wh, np.float32),
        b14=np.ascontiguousarray(
            np.stack([b1, bh[0], bh[1], bh[2]], axis=1), np.float32),
        b5rep=np.broadcast_to(np.asarray(bh[3], np.float32), (128, H)).copy(),
        woutrep=np.broadcast_to(np.asarray(Wout, np.float32)[:, 0], (128, H)).copy(),
    ), dict(bout=np.asarray(bout, np.float32))


# ---------------------------------------------------------------------------
# Bass program
# ---------------------------------------------------------------------------

def build_nc(T, B, S, weights):
    nc = bass.Bass("TRN2", target_bir_lowering=False)

    xT_d = nc.declare_dram_parameter("xT", [128, S], F32, isOutput=False)
    idx_d = nc.declare_dram_parameter("idx", [128, T * B], I32, isOutput=False)
    nrm_d = nc.declare_dram_parameter("nrm", [128, T * B], F32, isOutput=False)
    ldst_d = nc.declare_dram_parameter("ldst", [128, T * B], F32, isOutput=False)
    gcol_d = nc.declare_dram_parameter("gcol", [128, T], F32, isOutput=False)
    invcnt_d = nc.declare_dram_parameter("invcnt", [128, 1], F32, isOutput=False)
    iota_d = nc.declare_dram_parameter("iota", [128, 128], F32, isOutput=False)
    W1_d = nc.declare_dram_parameter("W1", [128, H], F32, isOutput=False)
    Wh_d = nc.declare_dram_parameter("Wh", [4, H, H], F32, isOutput=False)
    b14_d = nc.declare_dram_parameter("b14", [H, 4], F32, isOutput=False)
    b5rep_d = nc.declare_dram_parameter("b5rep", [128, H], F32, isOutput=False)
    woutrep_d = nc.declare_dram_parameter("woutrep", [128, H], F32, isOutput=False)
    out_d = nc.declare_dram_parameter("out", [128, 1], F32, isOutput=True)
    bout = float(weights["bout"][0])

    with tile.TileContext(nc) as tc:
        with contextlib.ExitStack() as ctx:
            zfull_pool = ctx.enter_context(tc.tile_pool(name="zfull", bufs=1, space="DRAM"))
            dram = ctx.enter_context(tc.tile_pool(name="dram", bufs=1, space="DRAM"))
            const = ctx.enter_context(tc.tile_pool(name="const", bufs=1))
            xp = ctx.enter_context(tc.tile_pool(name="xp", bufs=3))
            gp = ctx.enter_context(tc.tile_pool(name="gp", bufs=3))
            mp = ctx.enter_context(tc.tile_pool(name="mp", bufs=3))
            zp = ctx.enter_context(tc.tile_pool(name="zp", bufs=3))
            hp = ctx.enter_context(tc.tile_pool(name="hp", bufs=3))
            ep = ctx.enter_context(tc.tile_pool(name="ep", bufs=2))
            ps_agg = ctx.enter_context(tc.tile_pool(name="ps_agg", bufs=2, space="PSUM"))
            ps_z = ctx.enter_context(tc.tile_pool(name="ps_z", bufs=2, space="PSUM"))
            ps_misc = ctx.enter_context(tc.tile_pool(name="ps_misc", bufs=1, space="PSUM"))

            zfulls = [zfull_pool.tile([M * S, H], F32, addr_space="Shared",
                                      name=f"zfull{k}", tag=f"zfull{k}")
                      for k in range(5)]
            bounces = [dram.tile([S, H], F32, name=f"bounce{k}", tag=f"bounce{k}")
                       for k in range(5)]

            idx_sb = const.tile([128, T * B], I32)
            nc.sync.dma_start(idx_sb[:], idx_d[:])
            nrm_sb = const.tile([128, T * B], F32)
            nc.sync.dma_start(nrm_sb[:], nrm_d[:])
            ldst_sb = const.tile([128, T * B], F32)
            nc.sync.dma_start(ldst_sb[:], ldst_d[:])
            gcol_sb = const.tile([128, T], F32)
            nc.sync.dma_start(gcol_sb[:], gcol_d[:])
            invcnt_sb = const.tile([128, 1], F32)
            nc.sync.dma_start(invcnt_sb[:], invcnt_d[:])
            iota_sb = const.tile([128, 128], F32)
            nc.sync.dma_start(iota_sb[:], iota_d[:])
            W1_sb = const.tile([128, H], F32)
            nc.sync.dma_start(W1_sb[:], W1_d[:])
            Wh2_sb = const.tile([H, 4 * H], F32)
            for k in range(4):
                nc.sync.dma_start(Wh2_sb[:, k * H:(k + 1) * H], Wh_d[k, :, :])
            b14_sb = const.tile([H, 4], F32)
            nc.sync.dma_start(b14_sb[:], b14_d[:])
            b5rep_sb = const.tile([128, H], F32)
            nc.sync.dma_start(b5rep_sb[:], b5rep_d[:])
            woutrep_sb = const.tile([128, H], F32)
            nc.sync.dma_start(woutrep_sb[:], woutrep_d[:])
            ident = const.tile([H, H], F32)
            make_identity(nc, ident[:])

            # layer 0: z0 = x @ W1 for the local slice, then AllGather
            for t in range(T):
                xt = xp.tile([128, 128], F32, tag="xt")
                nc.sync.dma_start(xt[:], xT_d[:, t * 128:(t + 1) * 128])
                pz = ps_z.tile([128, H], F32, space="PSUM", tag="pz")
                nc.tensor.matmul(out=pz[:], lhsT=xt[:], rhs=W1_sb[:], start=True, stop=True)
                zt = zp.tile([128, H], F32, tag="zt")
                nc.scalar.copy(zt[:], pz[:])
                nc.sync.dma_start(bounces[0][t * 128:(t + 1) * 128, :], zt[:])
            nc.gpsimd.collective_compute(
                "AllGather", mybir.AluOpType.bypass,
                replica_groups=[list(range(M))],
                ins=[bounces[0][:]], outs=[zfulls[0][:]])

            # layers 1..5: gather z[src], scale, one-hot scatter matmul
            for layer in range(1, 6):
                pool_acc = None
                if layer == 5:
                    pool_acc = const.tile([H, 128], F32, name="pool_acc")
                    nc.vector.memset(pool_acc[:], 0.0)
                    h5all = const.tile([128, T * H], F32, name="h5all")
                for t in range(T):
                    g = gp.tile([128, B * 64], F32, tag="g")
                    for j in range(B):
                        # hw indirect DMA honors one offset per partition, so
                        # gather each 128-edge block with its own call
                        nc.gpsimd.indirect_dma_start(
                            out=g[:, j * 64:(j + 1) * 64], out_offset=None,
                            in_=zfulls[layer - 1][:],
                            in_offset=bass.IndirectOffsetOnAxis(
                                ap=idx_sb[:, t * B + j:t * B + j + 1], axis=0))
                    g3 = g[:].rearrange("p (b d) -> p b d", b=B)
                    nrm3 = nrm_sb[:, t * B:(t + 1) * B, None].to_broadcast([128, B, 64])
                    nc.vector.tensor_tensor(out=g3, in0=g3, in1=nrm3, op=mybir.AluOpType.mult)
                    m01 = mp.tile([128, B * 128], F32, tag="m01")
                    m3 = m01[:].rearrange("p (b q) -> p b q", b=B)
                    iota3 = iota_sb[:, None, :].to_broadcast([128, B, 128])
                    ldst3 = ldst_sb[:, t * B:(t + 1) * B, None].to_broadcast([128, B, 128])
                    nc.vector.tensor_tensor(out=m3, in0=iota3, in1=ldst3,
                                            op=mybir.AluOpType.is_equal)
                    if layer < 5:
                        pT = ps_agg.tile([H, 128], F32, space="PSUM", tag="pT")
                        for j in range(B):
                            nc.tensor.matmul(
                                out=pT[:], lhsT=g[:, j * 64:(j + 1) * 64],
                                rhs=m01[:, j * 128:(j + 1) * 128],
                                start=(j == 0), stop=(j == B - 1))
                        hT = hp.tile([H, 128], F32, tag="hT")
                        nc.scalar.activation(hT[:], pT[:], mybir.ActivationFunctionType.Relu,
                                             bias=b14_sb[:, layer - 1:layer])
                        pz = ps_z.tile([128, H], F32, space="PSUM", tag="pz")
                        nc.tensor.matmul(out=pz[:], lhsT=hT[:],
                                         rhs=Wh2_sb[:, (layer - 1) * H:layer * H],
                                         start=True, stop=True)
                        zt = zp.tile([128, H], F32, tag="zt2")
                        nc.scalar.copy(zt[:], pz[:])
                        nc.sync.dma_start(bounces[layer][t * 128:(t + 1) * 128, :], zt[:])
                    else:
                        p5 = ps_agg.tile([128, H], F32, space="PSUM", tag="pT")
                        for j in range(B):
                            nc.tensor.matmul(
                                out=p5[:], lhsT=m01[:, j * 128:(j + 1) * 128],
                                rhs=g[:, j * 64:(j + 1) * 64],
                                start=(j == 0), stop=(j == B - 1))
                        h5 = h5all[:, t * H:(t + 1) * H]
                        nc.vector.tensor_tensor(out=h5, in0=p5[:], in1=b5rep_sb[:],
                                                op=mybir.AluOpType.add)
                        nc.scalar.activation(h5, h5, mybir.ActivationFunctionType.Relu)
                if layer < 5:
                    nc.gpsimd.collective_compute(
                        "AllGather", mybir.AluOpType.bypass,
                        replica_groups=[list(range(M))],
                        ins=[bounces[layer][:]], outs=[zfulls[layer][:]])

            # mean-pool per graph (one-hot matmul per tile, accumulate in SBUF)
            for t in range(T):
                pt = mp.tile([128, 128], F32, tag="pt")
                nc.vector.tensor_tensor(
                    out=pt[:], in0=iota_sb[:],
                    in1=gcol_sb[:, t:t + 1].to_broadcast([128, 128]),
                    op=mybir.AluOpType.is_equal)
                ppool = ps_misc.tile([H, 128], F32, space="PSUM", tag="pool")
                nc.tensor.matmul(out=ppool[:], lhsT=h5all[:, t * H:(t + 1) * H],
                                 rhs=pt[:], start=True, stop=True)
                nc.vector.tensor_tensor(out=pool_acc[:], in0=pool_acc[:],
                                        in1=ppool[:], op=mybir.AluOpType.add)

            # transpose, scale by 1/cnt, layernorm, output head
            ptr = ps_z.tile([128, H], F32, space="PSUM", tag="pz")
            nc.tensor.transpose(out=ptr[:], in_=pool_acc[:], identity=ident[:])
            pooled = ep.tile([128, H], F32, tag="pooled")
            nc.vector.tensor_scalar(out=pooled[:], in0=ptr[:], scalar1=invcnt_sb[:, 0:1],
                                    scalar2=None, op0=mybir.AluOpType.mult)
            mu = ep.tile([128, 1], F32, tag="mu")
            nc.vector.tensor_reduce(out=mu[:], in_=pooled[:], axis=mybir.AxisListType.X,
                                    op=mybir.AluOpType.add)
            nc.vector.tensor_scalar(out=mu[:], in0=mu[:], scalar1=1.0 / H, scalar2=None,
                                    op0=mybir.AluOpType.mult)
            xc = ep.tile([128, H], F32, tag="xc")
            nc.vector.tensor_scalar(out=xc[:], in0=pooled[:], scalar1=mu[:, 0:1],
                                    scalar2=None, op0=mybir.AluOpType.subtract)
            sq = ep.tile([128, H], F32, tag="sq")
            nc.scalar.activation(sq[:], xc[:], mybir.ActivationFunctionType.Square)
            var = ep.tile([128, 1], F32, tag="var")
            nc.vector.tensor_reduce(out=var[:], in_=sq[:], axis=mybir.AxisListType.X,
                                    op=mybir.AluOpType.add)
            nc.vector.tensor_scalar(out=var[:], in0=var[:], scalar1=1.0 / H, scalar2=None,
                                    op0=mybir.AluOpType.mult)
            eps_col = ep.tile([128, 1], F32, tag="eps")
            nc.gpsimd.memset(eps_col[:], 1e-5)
            std = ep.tile([128, 1], F32, tag="std")
            nc.scalar.activation(std[:], var[:], mybir.ActivationFunctionType.Sqrt,
                                 bias=eps_col[:, 0:1])
            rstd = ep.tile([128, 1], F32, tag="rstd")
            nc.vector.reciprocal(rstd[:], std[:])
            ln = ep.tile([128, H], F32, tag="ln")
            nc.vector.tensor_scalar(out=ln[:], in0=xc[:], scalar1=rstd[:, 0:1],
                                    scalar2=None, op0=mybir.AluOpType.mult)
            y = ep.tile([128, H], F32, tag="y")
            nc.vector.tensor_tensor(out=y[:], in0=ln[:], in1=woutrep_sb[:],
                                    op=mybir.AluOpType.mult)
            yr = ep.tile([128, 1], F32, tag="yr")
            nc.vector.tensor_reduce(out=yr[:], in_=y[:], axis=mybir.AxisListType.X,
                                    op=mybir.AluOpType.add)
            nc.vector.tensor_scalar(out=yr[:], in0=yr[:], scalar1=bout, scalar2=None,
                                    op0=mybir.AluOpType.add)
            nc.sync.dma_start(out_d[:], yr[:])
    return nc


# ---------------------------------------------------------------------------
# Entry point
# ---------------------------------------------------------------------------

def kernel(x, edge_index, batch, W1, b1, Wh, bh, Wout, bout):
    from concourse.bass_utils import run_bass_kernel_spmd

    x = np.asarray(x, np.float32)
    edge_index = np.asarray(edge_index)
    batch = np.asarray(batch)
    n_graphs = 1000

    in_maps, meta = preprocess(x, edge_index, batch, n_graphs)
    wmaps, wmeta = make_weight_inputs(W1, b1, Wh, bh, Wout, bout)
    nc = build_nc(meta["T"], meta["B"], meta["S"], dict(bout=wmeta["bout"]))
    for im in in_maps:
        im.update(wmaps)

    import time
    last_err = None
    for attempt in range(3):
        try:
            res = run_bass_kernel_spmd(nc, in_maps, core_ids=list(range(M)))
            break
        except Exception as e:  # transient terminal hiccups / device recovery
            last_err = e
            time.sleep(30 * (attempt + 1))
    else:
        raise last_err

    GPC = meta["GPC"]
    out = np.concatenate([res.results[c]["out"][:GPC] for c in range(M)], axis=0)
    return np.ascontiguousarray(out, np.float32)



# revision 4
# speedup vs baseline: 1.2498x; 1.2498x over previous
"""GCN-5 message-passing kernel for Trainium2, 8-core SPMD Bass/Tile.

Strategy (graph-parallel):
  - batch is sorted, so graphs are contiguous node ranges.  Core c owns graphs
    [125c, 125(c+1)) and their nodes; edges are assigned to the core owning the
    dst node.  Pooling / layernorm / output head are fully local per core.
  - Per layer each core computes z' = dis * (h @ W) for its own node slice,
    writes it bf16 into a 256B-strided bounce buffer, AllGathers to zfull.
  - Edge gathering uses the batched InstDMAGatherAnt (dma_gather): edges are
    sorted by (tile-range, src-chunk, dst-tile) and padded to 128-blocks on
    the host; one gather call fetches all edges of a (range, chunk) in a
    single SWDGE instruction (int16 indices limit a chunk to 25600 rows).
  - Scatter-add per 128-node tile is a one-hot bf16 matmul with PSUM
    accumulation.  The symmetric norm dis[src]*dis[dst] is separable:
    dis[src] is folded into zfull at write time; dis[dst] is a per-edge
    scale applied in-place to the gathered data.  Self-loops never hit the
    gather path: they are one identity matmul per tile on dis^2-scaled z.
"""
import sys
import types
import contextlib

import numpy as np
import ml_dtypes

sys.path.insert(0, "/opt/trn_rl_repo")

import concourse.bass as bass
import concourse.tile as tile
from concourse import mybir
from concourse import library_config
from concourse.masks import make_identity
from concourse.vector_clock import ScopedClock

F32 = mybir.dt.float32
BF16 = mybir.dt.bfloat16
I16 = mybir.dt.int16
M = 8           # NeuronCores
H = 64
N = 100000
NG = 1000
GPC = NG // M   # graphs per core
T = 100         # node tiles per core
S = T * 128     # padded nodes per core
NSTOT = M * S   # padded rows in zfull
CH = 25600      # chunk stride (int16 gather index limit 32767)
NCH = 4
TR = 8          # tiles per gather range

# ---------------------------------------------------------------------------
# Environment fixes for this container (same as baseline)
# ---------------------------------------------------------------------------

def _install_env_fixes():
    import concourse.tile as tile_mod

    def _patched_drain_and_barrier(self, tick_clock, wait_clock):
        # this walrus build allows a single sync-wait per TPB_CTRL Drain;
        # split the Tile tail-drain's waits across multiple drains.
        nc = self.nc
        drain_inst = nc.sync.drain()
        wait_clock.add_sem_waits(drain_inst.ins,
                                 ScopedClock({None: tick_clock.global_clock}))
        si = drain_inst.ins.sync_info
        waits = list(si.on_wait or [])
        if len(waits) > 1:
            si.on_wait[:] = waits[:1]
            for w in waits[1:]:
                d2 = nc.sync.drain()
                if d2.ins.sync_info is None:
                    d2.ins.sync_info = mybir.SyncInfo(on_wait=[w], on_update=[])
                else:
                    d2.ins.sync_info.on_wait.append(w)
        nc.all_engine_barrier()
        assert self.sems is not None
        popped = nc._tile_sem_poison_stack.pop()
        assert popped is self._sem_poison
        nc.clear_and_free_semaphores(list(self.sems.allocated().values()))
        nc.all_engine_barrier()

    tile_mod.TileContext._drain_and_barrier = _patched_drain_and_barrier

    _orig_lower_ordered = tile_mod.TileContext._lower_ordered_insts

    def _split_multiwait_lower(self, ordered):
        nc = self.nc
        for bbname, insts in ordered.items():
            newlist = []
            changed = False
            for inst in insts:
                si = getattr(inst, "sync_info", None)
                eng = getattr(inst, "engine", None)
                if (si is not None and si.on_wait and len(si.on_wait) > 1
                        and eng is not None and eng != mybir.EngineType.Unassigned
                        and inst.is_executable()):
                    waits = list(si.on_wait)
                    si.on_wait[:] = waits[-1:]
                    for w in waits[:-1]:
                        nop = mybir.InstNoOp(
                            name=nc.get_next_instruction_name(), engine=eng)
                        nop.sync_info = mybir.SyncInfo(on_wait=[w], on_update=[])
                        try:
                            nc.register_instruction(nop, overwrite=True)
                        except Exception:
                            pass
                        newlist.append(nop)
                    changed = True
                newlist.append(inst)
            if changed:
                insts[:] = newlist
        return _orig_lower_ordered(self, ordered)

    if getattr(tile_mod.TileContext._lower_ordered_insts, "__name__", "") != \
            "_split_multiwait_lower":
        tile_mod.TileContext._lower_ordered_insts = _split_multiwait_lower

    # NTFF profile hook (lets trace=True work under axon); best-effort.
    if "antenv.axon_hooks" not in sys.modules:
        try:
            from trn_agent_boot.trn_boot import _ntff_profile_via_ctypes
            hook = _ntff_profile_via_ctypes("/opt/axon/libaxon_pjrt.so")
            mod = types.ModuleType("antenv.axon_hooks")
            mod.get_axon_ntff_profile_hook = lambda: hook
            mod.set_axon_ntff_profile_hook = lambda h: None
            sys.modules["antenv.axon_hooks"] = mod
            import antenv
            antenv.axon_hooks = mod
        except Exception:
            pass


_install_env_fixes()

# ---------------------------------------------------------------------------
# Host preprocessing
# ---------------------------------------------------------------------------

def _pack_idx(flat):
    """dma_gather index layout: tile[p, c] = flat[c*16 + p%16], replicated
    across the 8 groups of 16 partitions."""
    n = len(flat)
    a = np.ascontiguousarray(flat.astype(np.int16).reshape(n // 16, 16).T)
    return np.ascontiguousarray(np.tile(a, (8, 1)))


def preprocess(x, edge_index, batch):
    batch = np.asarray(batch)
    src = np.asarray(edge_index[0], np.int64)
    dst = np.asarray(edge_index[1], np.int64)

    node_start = np.searchsorted(batch, np.arange(M) * GPC)
    node_end = np.searchsorted(batch, np.arange(M) * GPC + GPC)
    n_c = node_end - node_start
    assert n_c.max() <= S

    owner = np.zeros(N, np.int64)
    for c in range(M):
        owner[node_start[c]:node_end[c]] = c
    local_pos = np.arange(N) - node_start[owner]
    row = owner * S + local_pos            # node's row in zfull

    deg = (np.bincount(dst, minlength=N) + 1).astype(np.float64)
    dis = (1.0 / np.sqrt(deg)).astype(np.float32)

    eo = owner[dst]
    # per-core sorted edge arrays
    pc = []
    cnt = np.zeros((M, T, NCH), np.int64)
    for c in range(M):
        sel = np.nonzero(eo == c)[0]
        es, ed = src[sel], dst[sel]
        ld = local_pos[ed]
        tid = ld // 128
        slot = ld % 128
        srow = row[es]
        ch = srow // CH
        rel = srow - ch * CH
        order = np.lexsort((ch, tid))
        es, ed, tid, slot, ch, rel = (a[order] for a in (es, ed, tid, slot, ch, rel))
        key = tid * NCH + ch
        cnt[c] = np.bincount(key, minlength=T * NCH).reshape(T, NCH)
        pc.append(dict(ed=ed, tid=tid, slot=slot, ch=ch, rel=rel, key=key))

    # uniform-across-cores block counts per (tile, chunk)
    B_tr = np.ceil(cnt.max(axis=0) / 128).astype(np.int64)     # [T, NCH]

    # ranges of tiles; global block order is (range, chunk, tile)
    ranges = [list(range(t0, min(t0 + TR, T))) for t0 in range(0, T, TR)]
    blk_off = np.zeros((T, NCH), np.int64)
    call_off = {}
    nb_call = {}
    off = 0
    for R, tiles in enumerate(ranges):
        for r in range(NCH):
            call_off[(R, r)] = off
            for t in tiles:
                blk_off[t, r] = off
                off += B_tr[t, r]
            raw = off - call_off[(R, r)]
            # quantize call sizes so few distinct num_idxs registers are used
            nb_call[(R, r)] = 0 if raw == 0 else int(np.ceil(raw / 4) * 4)
            off = call_off[(R, r)] + nb_call[(R, r)]
    NB = off
    range_off = [call_off[(R, 0)] for R in range(len(ranges))]
    range_nb = [sum(nb_call[(R, r)] for r in range(NCH)) for R in range(len(ranges))]

    # per-core flat edge-position arrays
    in_maps = []
    for c in range(M):
        d = pc[c]
        # rank of each edge within its (tile, chunk) group
        cc = cnt[c].reshape(-1)
        cum = np.concatenate([[0], np.cumsum(cc)])
        rank = np.arange(len(d["key"])) - cum[d["key"]]
        dest = blk_off[d["tid"], d["ch"]] * 128 + rank

        gidx = np.zeros(NB * 128, np.int64)
        ldst = np.full(NB * 128, -1.0, np.float32)
        disd = np.zeros(NB * 128, np.float32)
        gidx[dest] = d["rel"]
        ldst[dest] = d["slot"]
        disd[dest] = dis[d["ed"]]

        idx_t = _pack_idx(gidx)                                   # [128, NB*8]
        ldst_t = np.ascontiguousarray(
            ldst.reshape(NB, 128).T).astype(ml_dtypes.bfloat16)   # [128, NB]
        disd_t = np.ascontiguousarray(
            disd.reshape(NB, 128).T).astype(ml_dtypes.bfloat16)   # [128, NB]

        ns, ne = node_start[c], node_end[c]
        discol = np.zeros((128, T), np.float32)
        dv = dis[ns:ne]
        fl = np.zeros(S, np.float32)
        fl[:n_c[c]] = dv
        discol[:, :] = fl.reshape(T, 128).T
        dis2col = np.ascontiguousarray(discol * discol)

        gflat = np.full(S, -1.0, np.float32)
        gflat[:n_c[c]] = batch[ns:ne] - c * GPC
        gcol = np.ascontiguousarray(gflat.reshape(T, 128).T).astype(
            ml_dtypes.bfloat16)

        cnt_g = np.bincount(batch[ns:ne] - c * GPC, minlength=GPC).astype(np.float32)
        invcnt = np.zeros((128, 1), np.float32)
        invcnt[:GPC, 0] = 1.0 / np.maximum(cnt_g, 1.0)

        xT = np.zeros((128, S), ml_dtypes.bfloat16)
        xT[:, :n_c[c]] = np.asarray(x[ns:ne], np.float32).T.astype(
            ml_dtypes.bfloat16)

        in_maps.append(dict(
            xT=xT, idx=idx_t, ldst=ldst_t, disd=disd_t,
            discol=discol, dis2col=dis2col, gcol=gcol, invcnt=invcnt,
        ))

    iota = np.broadcast_to(np.arange(128, dtype=np.float32), (128, 128))
    common = dict(
        iota=np.ascontiguousarray(iota).astype(ml_dtypes.bfloat16),
        ident128=np.eye(128, dtype=np.float32).astype(ml_dtypes.bfloat16),
        ones1=np.ones((1, 128), ml_dtypes.bfloat16),
    )
    for im in in_maps:
        im.update(common)

    geom = dict(B_tr=B_tr, blk_off=blk_off, call_off=call_off, nb_call=nb_call,
                NB=NB, ranges=ranges, range_off=range_off, range_nb=range_nb)
    return in_maps, geom


def make_weight_inputs(W1, b1, Wh, bh, Wout, bout):
    W1 = np.asarray(W1, np.float32)
    Wh = np.asarray(Wh, np.float32)
    Wh2 = np.concatenate([Wh[k] for k in range(4)], axis=1)  # [64, 256]
    b14 = np.stack([np.asarray(b1, np.float32)] +
                   [np.asarray(bh[k], np.float32) for k in range(3)], axis=1)
    return dict(
        W1=W1.astype(ml_dtypes.bfloat16),
        Wh2=np.ascontiguousarray(Wh2).astype(ml_dtypes.bfloat16),
        b14=np.ascontiguousarray(b14),
        b5row=np.asarray(bh[3], np.float32).reshape(1, H).astype(
            ml_dtypes.bfloat16),
        woutrep=np.broadcast_to(np.asarray(Wout, np.float32)[:, 0],
                                (128, H)).copy(),
    ), dict(bout=float(np.asarray(bout).reshape(-1)[0]))


# ---------------------------------------------------------------------------
# Bass program
# ---------------------------------------------------------------------------

def build_nc(geom, bout):
    NB = geom["NB"]
    B_tr = geom["B_tr"]
    blk_off = geom["blk_off"]
    call_off = geom["call_off"]
    nb_call = geom["nb_call"]
    ranges = geom["ranges"]
    range_off = geom["range_off"]
    range_nb = geom["range_nb"]
    NBR = max(range_nb)

    nc = bass.Bass("TRN2", target_bir_lowering=False)

    xT_d = nc.declare_dram_parameter("xT", [128, S], BF16, isOutput=False)
    idx_d = nc.declare_dram_parameter("idx", [128, NB * 8], I16, isOutput=False)
    ldst_d = nc.declare_dram_parameter("ldst", [128, NB], BF16, isOutput=False)
    disd_d = nc.declare_dram_parameter("disd", [128, NB], BF16, isOutput=False)
    discol_d = nc.declare_dram_parameter("discol", [128, T], F32, isOutput=False)
    dis2col_d = nc.declare_dram_parameter("dis2col", [128, T], F32, isOutput=False)
    gcol_d = nc.declare_dram_parameter("gcol", [128, T], BF16, isOutput=False)
    invcnt_d = nc.declare_dram_parameter("invcnt", [128, 1], F32, isOutput=False)
    iota_d = nc.declare_dram_parameter("iota", [128, 128], BF16, isOutput=False)
    ident_d = nc.declare_dram_parameter("ident128", [128, 128], BF16, isOutput=False)
    ones1_d = nc.declare_dram_parameter("ones1", [1, 128], BF16, isOutput=False)
    W1_d = nc.declare_dram_parameter("W1", [128, H], BF16, isOutput=False)
    Wh2_d = nc.declare_dram_parameter("Wh2", [H, 4 * H], BF16, isOutput=False)
    b14_d = nc.declare_dram_parameter("b14", [H, 4], F32, isOutput=False)
    b5row_d = nc.declare_dram_parameter("b5row", [1, H], BF16, isOutput=False)
    woutrep_d = nc.declare_dram_parameter("woutrep", [128, H], F32, isOutput=False)
    out_d = nc.declare_dram_parameter("out", [128, 1], F32, isOutput=True)

    with tile.TileContext(nc) as tc:
        with contextlib.ExitStack() as ctx:
            dram = ctx.enter_context(tc.tile_pool(name="dram", bufs=1, space="DRAM"))
            zfull_pool = ctx.enter_context(
                tc.tile_pool(name="zfull", bufs=1, space="DRAM"))
            const = ctx.enter_context(tc.tile_pool(name="const", bufs=1))
            gp = ctx.enter_context(tc.tile_pool(name="gp", bufs=2))
            mp = ctx.enter_context(tc.tile_pool(name="mp", bufs=2))
            zsbp = ctx.enter_context(tc.tile_pool(name="zsbp", bufs=2))
            zst_p = ctx.enter_context(tc.tile_pool(name="zst", bufs=3))
            hTp = ctx.enter_context(tc.tile_pool(name="hTp", bufs=3))
            h5p = ctx.enter_context(tc.tile_pool(name="h5p", bufs=3))
            ptp = ctx.enter_context(tc.tile_pool(name="ptp", bufs=3))
            ep = ctx.enter_context(tc.tile_pool(name="ep", bufs=2))
            ps_agg = ctx.enter_context(tc.tile_pool(name="ps_agg", bufs=3, space="PSUM"))
            ps_z = ctx.enter_context(tc.tile_pool(name="ps_z", bufs=3, space="PSUM"))
            ps_pool = ctx.enter_context(tc.tile_pool(name="ps_pool", bufs=1, space="PSUM"))
            ps_misc = ctx.enter_context(tc.tile_pool(name="ps_misc", bufs=1, space="PSUM"))

            nc.gpsimd.load_library(library_config.mlp)

            zfulls = [zfull_pool.tile([NSTOT, 128], BF16, addr_space="Shared",
                                      name=f"zfull{k}", tag=f"zfull{k}")
                      for k in range(5)]
            bounces = [dram.tile([S, 128], BF16, name=f"bounce{k}", tag=f"bounce{k}")
                       for k in range(5)]

            idx_sb = const.tile([128, NB * 8], I16)
            nc.sync.dma_start(idx_sb[:], idx_d[:])
            ldst_sb = const.tile([128, NB], BF16)
            nc.sync.dma_start(ldst_sb[:], ldst_d[:])
            disd_sb = const.tile([128, NB], BF16)
            nc.sync.dma_start(disd_sb[:], disd_d[:])
            discol_sb = const.tile([128, T], F32)
            nc.sync.dma_start(discol_sb[:], discol_d[:])
            dis2col_sb = const.tile([128, T], F32)
            nc.sync.dma_start(dis2col_sb[:], dis2col_d[:])
            gcol_sb = const.tile([128, T], BF16)
            nc.sync.dma_start(gcol_sb[:], gcol_d[:])
            invcnt_sb = const.tile([128, 1], F32)
            nc.sync.dma_start(invcnt_sb[:], invcnt_d[:])
            iota_sb = const.tile([128, 128], BF16)
            nc.sync.dma_start(iota_sb[:], iota_d[:])
            ident_sb = const.tile([128, 128], BF16)
            nc.sync.dma_start(ident_sb[:], ident_d[:])
            ones1_sb = const.tile([1, 128], BF16)
            nc.sync.dma_start(ones1_sb[:], ones1_d[:])
            W1_sb = const.tile([128, H], BF16)
            nc.sync.dma_start(W1_sb[:], W1_d[:])
            Wh2_sb = const.tile([H, 4 * H], BF16)
            nc.sync.dma_start(Wh2_sb[:], Wh2_d[:])
            b14_sb = const.tile([H, 4], F32)
            nc.sync.dma_start(b14_sb[:], b14_d[:])
            b5row_sb = const.tile([1, H], BF16)
            nc.sync.dma_start(b5row_sb[:], b5row_d[:])
            woutrep_sb = const.tile([128, H], F32)
            nc.sync.dma_start(woutrep_sb[:], woutrep_d[:])
            xT_sb = const.tile([128, S], BF16)
            nc.sync.dma_start(xT_sb[:], xT_d[:])
            identH = const.tile([H, H], F32)
            make_identity(nc, identH[:])

            def write_z(layer, pz, t, zst, zsb):
                """pz [128,64] psum -> zst (dis*z bf16, DMA-staged) and
                zsb (dis^2*z bf16, for next layer's self-loop)."""
                g = t % 4
                nc.scalar.activation(
                    zst[:, g, :], pz[:],
                    mybir.ActivationFunctionType.Copy,
                    scale=discol_sb[:, t:t + 1])
                nc.vector.tensor_scalar(
                    out=zsb[:, t * H:(t + 1) * H], in0=pz[:],
                    scalar1=dis2col_sb[:, t:t + 1], scalar2=None,
                    op0=mybir.AluOpType.mult)
                if g == 3 or t == T - 1:
                    t0 = t - g
                    bo = bounces[layer][:].rearrange(
                        "(t p) d -> p t d", p=128)
                    nc.sync.dma_start(
                        bo[:, t0:t + 1, 0:H], zst[:, 0:g + 1, :])
                    # fill the 256B-row padding too (keeps zfull finite)
                    nc.sync.dma_start(
                        bo[:, t0:t + 1, H:2 * H], zst[:, 0:g + 1, :])

            # ---------------- layer 0: z0 = dis * (x @ W1) ----------------
            zsb = zsbp.tile([128, T * H], BF16, tag="zsb")
            zst = None
            for t in range(T):
                if t % 4 == 0:
                    zst = zst_p.tile([128, 4, H], BF16, tag="zst")
                pz = ps_z.tile([128, H], F32, space="PSUM", tag="pz")
                nc.tensor.matmul(out=pz[:], lhsT=xT_sb[:, t * 128:(t + 1) * 128],
                                 rhs=W1_sb[:], start=True, stop=True)
                write_z(0, pz, t, zst, zsb)
            nc.gpsimd.collective_compute(
                "AllGather", mybir.AluOpType.bypass,
                replica_groups=[list(range(M))],
                ins=[bounces[0][:]], outs=[zfulls[0][:]])

            # ---------------- layers 1..5 ----------------
            reg_cache = {}

            def nidx_reg(n):
                if n not in reg_cache:
                    reg_cache[n] = nc.gpsimd.to_reg(n)
                return reg_cache[n]

            for layer in range(1, 6):
                zsb_prev = zsb
                if layer < 5:
                    zsb = zsbp.tile([128, T * H], BF16, tag="zsb")
                else:
                    ppool = ps_pool.tile([H, 128], F32, space="PSUM", tag="ppool")
                zf = zfulls[layer - 1]
                for R, tiles in enumerate(ranges):
                    nbR = range_nb[R]
                    oR = range_off[R]
                    gR = gp.tile([128, NBR, 128], BF16, tag="gR")
                    for r in range(NCH):
                        nb = nb_call[(R, r)]
                        if nb == 0:
                            continue
                        o = call_off[(R, r)] - oR
                        rows = min(CH, NSTOT - r * CH)
                        nc.gpsimd.dma_gather(
                            gR[:, o:o + nb, :],
                            zf[r * CH:r * CH + rows, :],
                            idx_sb[:, call_off[(R, r)] * 8:
                                   (call_off[(R, r)] + nb) * 8],
                            nb * 128, nidx_reg(nb * 128), 128,
                            single_packet=False)
                    # one-hot masks for the whole range; dis[dst] onto g
                    mR = mp.tile([128, NBR, 128], BF16, tag="mR")
                    nc.vector.tensor_tensor(
                        out=mR[:, 0:nbR, :],
                        in0=iota_sb[:, None, :].to_broadcast([128, nbR, 128]),
                        in1=ldst_sb[:, oR:oR + nbR, None].to_broadcast(
                            [128, nbR, 128]),
                        op=mybir.AluOpType.is_equal)
                    nc.vector.tensor_tensor(
                        out=gR[:, 0:nbR, 0:H],
                        in0=gR[:, 0:nbR, 0:H],
                        in1=disd_sb[:, oR:oR + nbR, None].to_broadcast(
                            [128, nbR, H]),
                        op=mybir.AluOpType.mult)
                    for t in tiles:
                        if layer < 5:
                            pagg = ps_agg.tile([H, 128], F32, space="PSUM",
                                               tag="pagg")
                            first = True
                            for r in range(NCH):
                                o = blk_off[t, r] - oR
                                for b in range(B_tr[t, r]):
                                    nc.tensor.matmul(
                                        out=pagg[:],
                                        lhsT=gR[:, o + b, 0:H],
                                        rhs=mR[:, o + b, :],
                                        start=first, stop=False)
                                    first = False
                            nc.tensor.matmul(
                                out=pagg[:],
                                lhsT=zsb_prev[:, t * H:(t + 1) * H],
                                rhs=ident_sb[:],
                                start=first, stop=True)
                            hT = hTp.tile([H, 128], BF16, tag="hT")
                            nc.scalar.activation(
                                hT[:], pagg[:],
                                mybir.ActivationFunctionType.Relu,
                                bias=b14_sb[:, layer - 1:layer])
                            if t % 4 == 0:
                                zst = zst_p.tile([128, 4, H], BF16, tag="zst")
                            pz = ps_z.tile([128, H], F32, space="PSUM", tag="pz")
                            nc.tensor.matmul(
                                out=pz[:], lhsT=hT[:],
                                rhs=Wh2_sb[:, (layer - 1) * H:layer * H],
                                start=True, stop=True)
                            write_z(layer, pz, t, zst, zsb)
                        else:
                            p5 = ps_agg.tile([128, H], F32, space="PSUM",
                                             tag="pagg")
                            first = True
                            for r in range(NCH):
                                o = blk_off[t, r] - oR
                                for b in range(B_tr[t, r]):
                                    nc.tensor.matmul(
                                        out=p5[:],
                                        lhsT=mR[:, o + b, :],
                                        rhs=gR[:, o + b, 0:H],
                                        start=first, stop=False)
                                    first = False
                            nc.tensor.matmul(
                                out=p5[:], lhsT=ident_sb[:],
                                rhs=zsb_prev[:, t * H:(t + 1) * H],
                                start=first, stop=False)
                            nc.tensor.matmul(
                                out=p5[:], lhsT=ones1_sb[:], rhs=b5row_sb[:],
                                start=False, stop=True)
                            h5 = h5p.tile([128, H], BF16, tag="h5")
                            nc.scalar.activation(
                                h5[:], p5[:], mybir.ActivationFunctionType.Relu)
                            pt = ptp.tile([128, 128], BF16, tag="pt")
                            nc.vector.tensor_tensor(
                                out=pt[:], in0=iota_sb[:],
                                in1=gcol_sb[:, t:t + 1].to_broadcast([128, 128]),
                                op=mybir.AluOpType.is_equal)
                            nc.tensor.matmul(
                                out=ppool[:], lhsT=h5[:], rhs=pt[:],
                                start=(t == 0), stop=(t == T - 1))
                if layer < 5:
                    nc.gpsimd.collective_compute(
                        "AllGather", mybir.AluOpType.bypass,
                        replica_groups=[list(range(M))],
                        ins=[bounces[layer][:]], outs=[zfulls[layer][:]])

            # ---------------- mean-pool, layernorm, head ----------------
            poolT = ep.tile([H, 128], F32, tag="poolT")
            nc.scalar.copy(poolT[:], ppool[:])
            ptr = ps_misc.tile([128, H], F32, space="PSUM", tag="ptr")
            nc.tensor.transpose(out=ptr[:], in_=poolT[:], identity=identH[:])
            pooled = ep.tile([128, H], F32, tag="pooled")
            nc.vector.tensor_scalar(out=pooled[:], in0=ptr[:],
                                    scalar1=invcnt_sb[:, 0:1],
                                    scalar2=None, op0=mybir.AluOpType.mult)
            mu = ep.tile([128, 1], F32, tag="mu")
            nc.vector.tensor_reduce(out=mu[:], in_=pooled[:],
                                    axis=mybir.AxisListType.X,
                                    op=mybir.AluOpType.add)
            nc.vector.tensor_scalar(out=mu[:], in0=mu[:], scalar1=1.0 / H,
                                    scalar2=None, op0=mybir.AluOpType.mult)
            xc = ep.tile([128, H], F32, tag="xc")
            nc.vector.tensor_scalar(out=xc[:], in0=pooled[:],
                                    scalar1=mu[:, 0:1],
                                    scalar2=None, op0=mybir.AluOpType.subtract)
            sq = ep.tile([128, H], F32, tag="sq")
            nc.scalar.activation(sq[:], xc[:],
                                 mybir.ActivationFunctionType.Square)
            var = ep.tile([128, 1], F32, tag="var")
            nc.vector.tensor_reduce(out=var[:], in_=sq[:],
                                    axis=mybir.AxisListType.X,
                                    op=mybir.AluOpType.add)
            nc.vector.tensor_scalar(out=var[:], in0=var[:], scalar1=1.0 / H,
                                    scalar2=None, op0=mybir.AluOpType.mult)
            eps_col = ep.tile([128, 1], F32, tag="eps")
            nc.gpsimd.memset(eps_col[:], 1e-5)
            std = ep.tile([128, 1], F32, tag="std")
            nc.scalar.activation(std[:], var[:],
                                 mybir.ActivationFunctionType.Sqrt,
                                 bias=eps_col[:, 0:1])
            rstd = ep.tile([128, 1], F32, tag="rstd")
            nc.vector.reciprocal(rstd[:], std[:])
            ln = ep.tile([128, H], F32, tag="ln")
            nc.vector.tensor_scalar(out=ln[:], in0=xc[:], scalar1=rstd[:, 0:1],
                                    scalar2=None, op0=mybir.AluOpType.mult)
            y = ep.tile([128, H], F32, tag="y")
            nc.vector.tensor_tensor(out=y[:], in0=ln[:], in1=woutrep_sb[:],
                                    op=mybir.AluOpType.mult)
            yr = ep.tile([128, 1], F32, tag="yr")
            nc.vector.tensor_reduce(out=yr[:], in_=y[:],
                                    axis=mybir.AxisListType.X,
                                    op=mybir.AluOpType.add)
            nc.vector.tensor_scalar(out=yr[:], in0=yr[:], scalar1=bout,
                                    scalar2=None, op0=mybir.AluOpType.add)
            nc.sync.dma_start(out_d[:], yr[:])

    mybir.codegen_inst_isa_subclasses(nc)
    return nc


# ---------------------------------------------------------------------------
# Entry point
# ---------------------------------------------------------------------------

def kernel(x, edge_index, batch, W1, b1, Wh, bh, Wout, bout):
    from concourse.bass_utils import run_bass_kernel_spmd

    x = np.asarray(x, np.float32)
    in_maps, geom = preprocess(x, edge_index, batch)
    wmaps, wmeta = make_weight_inputs(W1, b1, Wh, bh, Wout, bout)
    for im in in_maps:
        im.update(wmaps)
    nc = build_nc(geom, wmeta["bout"])

    import time
    last_err = None
    for attempt in range(3):
        try:
            res = run_bass_kernel_spmd(nc, in_maps, core_ids=list(range(M)))
            break
        except Exception as e:  # transient terminal hiccups / device recovery
            last_err = e
            time.sleep(30 * (attempt + 1))
    else:
        raise last_err

    out = np.concatenate([res.results[c]["out"][:GPC] for c in range(M)], axis=0)
    return np.ascontiguousarray(out, np.float32)


# revision 6
# speedup vs baseline: 2.4043x; 1.9238x over previous
"""GCN-5 message-passing kernel for Trainium2, 8-core SPMD Bass/Tile.

Strategy (graph-parallel):
  - batch is sorted, so graphs are contiguous node ranges.  Core c owns graphs
    [125c, 125(c+1)) and their nodes; edges are assigned to the core owning the
    dst node.  Pooling / layernorm / output head are fully local per core.
  - Per layer each core computes z' = dis * (h @ W) for its own node slice,
    writes it bf16 into a 256B-strided bounce buffer, AllGathers to zfull.
  - Edge gathering uses the batched InstDMAGatherAnt (dma_gather): edges are
    sorted by (tile-range, src-chunk, dst-tile) and padded to 128-blocks on
    the host; one gather call fetches all edges of a (range, chunk) in a
    single SWDGE instruction (int16 indices limit a chunk to 25600 rows).
  - Scatter-add per 128-node tile is a one-hot bf16 matmul with PSUM
    accumulation.  The symmetric norm dis[src]*dis[dst] is separable:
    dis[src] is folded into zfull at write time; dis[dst] is a per-edge
    scale applied in-place to the gathered data.  Self-loops never hit the
    gather path: they are one identity matmul per tile on dis^2-scaled z.
"""
import sys
import types
import contextlib

import numpy as np
import ml_dtypes

sys.path.insert(0, "/opt/trn_rl_repo")

import concourse.bass as bass
import concourse.tile as tile
from concourse import mybir
from concourse import library_config
from concourse.masks import make_identity
from concourse.vector_clock import ScopedClock

F32 = mybir.dt.float32
BF16 = mybir.dt.bfloat16
I16 = mybir.dt.int16
M = 8           # NeuronCores
H = 64
N = 100000
NG = 1000
GPC = NG // M   # graphs per core
T = 100         # node tiles per core
S = T * 128     # padded nodes per core
NSTOT = M * S   # padded rows in zfull
CH = 25600      # chunk stride (int16 gather index limit 32767)
NCH = 4
TR = 8          # tiles per gather range

# ---------------------------------------------------------------------------
# Environment fixes for this container (same as baseline)
# ---------------------------------------------------------------------------

def _install_env_fixes():
    import concourse.tile as tile_mod

    def _patched_drain_and_barrier(self, tick_clock, wait_clock):
        # this walrus build allows a single sync-wait per TPB_CTRL Drain;
        # split the Tile tail-drain's waits across multiple drains.
        nc = self.nc
        drain_inst = nc.sync.drain()
        wait_clock.add_sem_waits(drain_inst.ins,
                                 ScopedClock({None: tick_clock.global_clock}))
        si = drain_inst.ins.sync_info
        waits = list(si.on_wait or [])
        if len(waits) > 1:
            si.on_wait[:] = waits[:1]
            for w in waits[1:]:
                d2 = nc.sync.drain()
                if d2.ins.sync_info is None:
                    d2.ins.sync_info = mybir.SyncInfo(on_wait=[w], on_update=[])
                else:
                    d2.ins.sync_info.on_wait.append(w)
        nc.all_engine_barrier()
        assert self.sems is not None
        popped = nc._tile_sem_poison_stack.pop()
        assert popped is self._sem_poison
        nc.clear_and_free_semaphores(list(self.sems.allocated().values()))
        nc.all_engine_barrier()

    tile_mod.TileContext._drain_and_barrier = _patched_drain_and_barrier

    _orig_lower_ordered = tile_mod.TileContext._lower_ordered_insts

    def _split_multiwait_lower(self, ordered):
        nc = self.nc
        for bbname, insts in ordered.items():
            newlist = []
            changed = False
            for inst in insts:
                si = getattr(inst, "sync_info", None)
                eng = getattr(inst, "engine", None)
                if (si is not None and si.on_wait and len(si.on_wait) > 1
                        and eng is not None and eng != mybir.EngineType.Unassigned
                        and inst.is_executable()):
                    waits = list(si.on_wait)
                    si.on_wait[:] = waits[-1:]
                    for w in waits[:-1]:
                        nop = mybir.InstNoOp(
                            name=nc.get_next_instruction_name(), engine=eng)
                        nop.sync_info = mybir.SyncInfo(on_wait=[w], on_update=[])
                        try:
                            nc.register_instruction(nop, overwrite=True)
                        except Exception:
                            pass
                        newlist.append(nop)
                    changed = True
                newlist.append(inst)
            if changed:
                insts[:] = newlist
        return _orig_lower_ordered(self, ordered)

    if getattr(tile_mod.TileContext._lower_ordered_insts, "__name__", "") != \
            "_split_multiwait_lower":
        tile_mod.TileContext._lower_ordered_insts = _split_multiwait_lower

    # Queue-aware DMASW sem lanes: Tile assigns SWDGE completion sems
    # round-robin in *scheduled* order, which mixes queues within a sem lane;
    # completions across queues reorder, breaking the lane's FIFO-threshold
    # semantics.  Pin each SWDGE queue to its own pair of lanes instead.
    import concourse.tile_sem_assignment as tsa

    if not getattr(tsa.TileClockTick, "_queue_lane_patch", False):
        _orig_assign_tick = tsa.TileClockTick._assign_tick

        def _queue_aware_assign_tick(self, inst):
            qn_ = getattr(inst, "queue_num", None)
            if (qn_ is not None and inst.engine == mybir.EngineType.Pool
                    and isinstance(inst, tsa.DMAInst)):
                tog = getattr(self, "_queue_lane_toggle", None)
                if tog is None:
                    tog = self._queue_lane_toggle = [0, 0, 0, 0]
                self.next_sw_dma_idx = (qn_ * 2 + tog[qn_]) % self.swdge_sem_count
                tog[qn_] ^= 1
            return _orig_assign_tick(self, inst)

        tsa.TileClockTick._assign_tick = _queue_aware_assign_tick
        tsa.TileClockTick._queue_lane_patch = True

    # NTFF profile hook (lets trace=True work under axon); best-effort.
    if "antenv.axon_hooks" not in sys.modules:
        try:
            from trn_agent_boot.trn_boot import _ntff_profile_via_ctypes
            hook = _ntff_profile_via_ctypes("/opt/axon/libaxon_pjrt.so")
            mod = types.ModuleType("antenv.axon_hooks")
            mod.get_axon_ntff_profile_hook = lambda: hook
            mod.set_axon_ntff_profile_hook = lambda h: None
            sys.modules["antenv.axon_hooks"] = mod
            import antenv
            antenv.axon_hooks = mod
        except Exception:
            pass


_install_env_fixes()

# ---------------------------------------------------------------------------
# Host preprocessing
# ---------------------------------------------------------------------------

def _pack_idx(flat):
    """dma_gather index layout: tile[p, c] = flat[c*16 + p%16], replicated
    across the 8 groups of 16 partitions."""
    n = len(flat)
    a = np.ascontiguousarray(flat.astype(np.int16).reshape(n // 16, 16).T)
    return np.ascontiguousarray(np.tile(a, (8, 1)))


def preprocess(x, edge_index, batch):
    batch = np.asarray(batch)
    src = np.asarray(edge_index[0], np.int64)
    dst = np.asarray(edge_index[1], np.int64)

    node_start = np.searchsorted(batch, np.arange(M) * GPC)
    node_end = np.searchsorted(batch, np.arange(M) * GPC + GPC)
    n_c = node_end - node_start
    assert n_c.max() <= S

    owner = np.zeros(N, np.int64)
    for c in range(M):
        owner[node_start[c]:node_end[c]] = c
    local_pos = np.arange(N) - node_start[owner]
    row = owner * S + local_pos            # node's row in zfull

    deg = (np.bincount(dst, minlength=N) + 1).astype(np.float64)
    dis = (1.0 / np.sqrt(deg)).astype(np.float32)

    eo = owner[dst]
    # per-core sorted edge arrays
    pc = []
    cnt = np.zeros((M, T, NCH), np.int64)
    for c in range(M):
        sel = np.nonzero(eo == c)[0]
        es, ed = src[sel], dst[sel]
        ld = local_pos[ed]
        tid = ld // 128
        slot = ld % 128
        srow = row[es]
        ch = srow // CH
        rel = srow - ch * CH
        order = np.lexsort((ch, tid))
        es, ed, tid, slot, ch, rel = (a[order] for a in (es, ed, tid, slot, ch, rel))
        key = tid * NCH + ch
        cnt[c] = np.bincount(key, minlength=T * NCH).reshape(T, NCH)
        pc.append(dict(ed=ed, tid=tid, slot=slot, ch=ch, rel=rel, key=key))

    # uniform-across-cores block counts per (tile, chunk)
    B_tr = np.ceil(cnt.max(axis=0) / 128).astype(np.int64)     # [T, NCH]

    # ranges of tiles; global block order is (range, chunk, tile)
    ranges = [list(range(t0, min(t0 + TR, T))) for t0 in range(0, T, TR)]
    blk_off = np.zeros((T, NCH), np.int64)
    call_off = {}
    nb_call = {}
    off = 0
    for R, tiles in enumerate(ranges):
        for r in range(NCH):
            call_off[(R, r)] = off
            for t in tiles:
                blk_off[t, r] = off
                off += B_tr[t, r]
            raw = off - call_off[(R, r)]
            # quantize call sizes so few distinct num_idxs registers are used
            nb_call[(R, r)] = 0 if raw == 0 else int(np.ceil(raw / 4) * 4)
            off = call_off[(R, r)] + nb_call[(R, r)]
    NB = off
    range_off = [call_off[(R, 0)] for R in range(len(ranges))]
    range_nb = [sum(nb_call[(R, r)] for r in range(NCH)) for R in range(len(ranges))]

    # per-core flat edge-position arrays
    in_maps = []
    for c in range(M):
        d = pc[c]
        # rank of each edge within its (tile, chunk) group
        cc = cnt[c].reshape(-1)
        cum = np.concatenate([[0], np.cumsum(cc)])
        rank = np.arange(len(d["key"])) - cum[d["key"]]
        dest = blk_off[d["tid"], d["ch"]] * 128 + rank

        gidx = np.zeros(NB * 128, np.int64)
        ldst = np.full(NB * 128, -1.0, np.float32)
        disd = np.zeros(NB * 128, np.float32)
        gidx[dest] = d["rel"]
        ldst[dest] = d["slot"]
        disd[dest] = dis[d["ed"]]

        idx_t = _pack_idx(gidx)                                   # [128, NB*8]
        ldst_t = np.ascontiguousarray(
            ldst.reshape(NB, 128).T).astype(ml_dtypes.bfloat16)   # [128, NB]
        disd_t = np.ascontiguousarray(
            disd.reshape(NB, 128).T).astype(ml_dtypes.bfloat16)   # [128, NB]

        ns, ne = node_start[c], node_end[c]
        discol = np.zeros((128, T), np.float32)
        dv = dis[ns:ne]
        fl = np.zeros(S, np.float32)
        fl[:n_c[c]] = dv
        discol[:, :] = fl.reshape(T, 128).T
        dis2col = np.ascontiguousarray(discol * discol)

        gflat = np.full(S, -1.0, np.float32)
        gflat[:n_c[c]] = batch[ns:ne] - c * GPC
        gcol = np.ascontiguousarray(gflat.reshape(T, 128).T).astype(
            ml_dtypes.bfloat16)

        cnt_g = np.bincount(batch[ns:ne] - c * GPC, minlength=GPC).astype(np.float32)
        invcnt = np.zeros((128, 1), np.float32)
        invcnt[:GPC, 0] = 1.0 / np.maximum(cnt_g, 1.0)

        xT = np.zeros((128, S), ml_dtypes.bfloat16)
        xT[:, :n_c[c]] = np.asarray(x[ns:ne], np.float32).T.astype(
            ml_dtypes.bfloat16)

        in_maps.append(dict(
            xT=xT, idx=idx_t, ldst=ldst_t, disd=disd_t,
            discol=discol, dis2col=dis2col, gcol=gcol, invcnt=invcnt,
        ))

    iota = np.broadcast_to(np.arange(128, dtype=np.float32), (128, 128))
    common = dict(
        iota=np.ascontiguousarray(iota).astype(ml_dtypes.bfloat16),
        ident128=np.eye(128, dtype=np.float32).astype(ml_dtypes.bfloat16),
        ones1=np.ones((1, 128), ml_dtypes.bfloat16),
    )
    for im in in_maps:
        im.update(common)

    geom = dict(B_tr=B_tr, blk_off=blk_off, call_off=call_off, nb_call=nb_call,
                NB=NB, ranges=ranges, range_off=range_off, range_nb=range_nb)
    return in_maps, geom


def make_weight_inputs(W1, b1, Wh, bh, Wout, bout):
    W1 = np.asarray(W1, np.float32)
    Wh = np.asarray(Wh, np.float32)
    Wh2 = np.concatenate([Wh[k] for k in range(4)], axis=1)  # [64, 256]
    b14 = np.stack([np.asarray(b1, np.float32)] +
                   [np.asarray(bh[k], np.float32) for k in range(3)], axis=1)
    return dict(
        W1=W1.astype(ml_dtypes.bfloat16),
        Wh2=np.ascontiguousarray(Wh2).astype(ml_dtypes.bfloat16),
        b14=np.ascontiguousarray(b14),
        b5row=np.asarray(bh[3], np.float32).reshape(1, H).astype(
            ml_dtypes.bfloat16),
        woutrep=np.broadcast_to(np.asarray(Wout, np.float32)[:, 0],
                                (128, H)).copy(),
    ), dict(bout=float(np.asarray(bout).reshape(-1)[0]))


# ---------------------------------------------------------------------------
# Bass program
# ---------------------------------------------------------------------------

def build_nc(geom, bout):
    NB = geom["NB"]
    B_tr = geom["B_tr"]
    blk_off = geom["blk_off"]
    call_off = geom["call_off"]
    nb_call = geom["nb_call"]
    ranges = geom["ranges"]
    range_off = geom["range_off"]
    range_nb = geom["range_nb"]
    NBR = max(range_nb)

    nc = bass.Bass("TRN2", target_bir_lowering=False, num_swdge_queues=4)

    xT_d = nc.declare_dram_parameter("xT", [128, S], BF16, isOutput=False)
    idx_d = nc.declare_dram_parameter("idx", [128, NB * 8], I16, isOutput=False)
    ldst_d = nc.declare_dram_parameter("ldst", [128, NB], BF16, isOutput=False)
    disd_d = nc.declare_dram_parameter("disd", [128, NB], BF16, isOutput=False)
    discol_d = nc.declare_dram_parameter("discol", [128, T], F32, isOutput=False)
    dis2col_d = nc.declare_dram_parameter("dis2col", [128, T], F32, isOutput=False)
    gcol_d = nc.declare_dram_parameter("gcol", [128, T], BF16, isOutput=False)
    invcnt_d = nc.declare_dram_parameter("invcnt", [128, 1], F32, isOutput=False)
    iota_d = nc.declare_dram_parameter("iota", [128, 128], BF16, isOutput=False)
    ident_d = nc.declare_dram_parameter("ident128", [128, 128], BF16, isOutput=False)
    ones1_d = nc.declare_dram_parameter("ones1", [1, 128], BF16, isOutput=False)
    W1_d = nc.declare_dram_parameter("W1", [128, H], BF16, isOutput=False)
    Wh2_d = nc.declare_dram_parameter("Wh2", [H, 4 * H], BF16, isOutput=False)
    b14_d = nc.declare_dram_parameter("b14", [H, 4], F32, isOutput=False)
    b5row_d = nc.declare_dram_parameter("b5row", [1, H], BF16, isOutput=False)
    woutrep_d = nc.declare_dram_parameter("woutrep", [128, H], F32, isOutput=False)
    out_d = nc.declare_dram_parameter("out", [128, 1], F32, isOutput=True)

    with tile.TileContext(nc) as tc:
        with contextlib.ExitStack() as ctx:
            dram = ctx.enter_context(tc.tile_pool(name="dram", bufs=1, space="DRAM"))
            zfull_pool = ctx.enter_context(
                tc.tile_pool(name="zfull", bufs=1, space="DRAM"))
            const = ctx.enter_context(tc.tile_pool(name="const", bufs=1))
            gp = ctx.enter_context(tc.tile_pool(name="gp", bufs=2))
            mp = ctx.enter_context(tc.tile_pool(name="mp", bufs=2))
            zsbp = ctx.enter_context(tc.tile_pool(name="zsbp", bufs=2))
            zst_p = ctx.enter_context(tc.tile_pool(name="zst", bufs=3))
            hTp = ctx.enter_context(tc.tile_pool(name="hTp", bufs=3))
            h5p = ctx.enter_context(tc.tile_pool(name="h5p", bufs=3))
            ptp = ctx.enter_context(tc.tile_pool(name="ptp", bufs=3))
            ep = ctx.enter_context(tc.tile_pool(name="ep", bufs=2))
            ps_agg = ctx.enter_context(tc.tile_pool(name="ps_agg", bufs=3, space="PSUM"))
            ps_z = ctx.enter_context(tc.tile_pool(name="ps_z", bufs=3, space="PSUM"))
            ps_pool = ctx.enter_context(tc.tile_pool(name="ps_pool", bufs=1, space="PSUM"))
            ps_misc = ctx.enter_context(tc.tile_pool(name="ps_misc", bufs=1, space="PSUM"))

            nc.gpsimd.load_library(library_config.mlp)

            zfulls = [zfull_pool.tile([NSTOT, 128], BF16, addr_space="Shared",
                                      name=f"zfull{k}", tag=f"zfull{k}")
                      for k in range(5)]
            bounces = [dram.tile([S, 128], BF16, name=f"bounce{k}", tag=f"bounce{k}")
                       for k in range(5)]

            idx_sb = const.tile([128, NB * 8], I16)
            nc.sync.dma_start(idx_sb[:], idx_d[:])
            ldst_sb = const.tile([128, NB], BF16)
            nc.sync.dma_start(ldst_sb[:], ldst_d[:])
            disd_sb = const.tile([128, NB], BF16)
            nc.sync.dma_start(disd_sb[:], disd_d[:])
            discol_sb = const.tile([128, T], F32)
            nc.sync.dma_start(discol_sb[:], discol_d[:])
            dis2col_sb = const.tile([128, T], F32)
            nc.sync.dma_start(dis2col_sb[:], dis2col_d[:])
            gcol_sb = const.tile([128, T], BF16)
            nc.sync.dma_start(gcol_sb[:], gcol_d[:])
            invcnt_sb = const.tile([128, 1], F32)
            nc.sync.dma_start(invcnt_sb[:], invcnt_d[:])
            iota_sb = const.tile([128, 128], BF16)
            nc.sync.dma_start(iota_sb[:], iota_d[:])
            ident_sb = const.tile([128, 128], BF16)
            nc.sync.dma_start(ident_sb[:], ident_d[:])
            ones1_sb = const.tile([1, 128], BF16)
            nc.sync.dma_start(ones1_sb[:], ones1_d[:])
            W1_sb = const.tile([128, H], BF16)
            nc.sync.dma_start(W1_sb[:], W1_d[:])
            Wh2_sb = const.tile([H, 4 * H], BF16)
            nc.sync.dma_start(Wh2_sb[:], Wh2_d[:])
            b14_sb = const.tile([H, 4], F32)
            nc.sync.dma_start(b14_sb[:], b14_d[:])
            b5row_sb = const.tile([1, H], BF16)
            nc.sync.dma_start(b5row_sb[:], b5row_d[:])
            woutrep_sb = const.tile([128, H], F32)
            nc.sync.dma_start(woutrep_sb[:], woutrep_d[:])
            xT_sb = const.tile([128, S], BF16)
            nc.sync.dma_start(xT_sb[:], xT_d[:])
            identH = const.tile([H, H], F32)
            make_identity(nc, identH[:])

            def write_z(layer, pz, t, zst, zsb):
                """pz [128,64] psum -> zst (dis*z bf16, DMA-staged) and
                zsb (dis^2*z bf16, for next layer's self-loop)."""
                g = t % 4
                nc.scalar.activation(
                    zst[:, g, :], pz[:],
                    mybir.ActivationFunctionType.Copy,
                    scale=discol_sb[:, t:t + 1])
                nc.vector.tensor_scalar(
                    out=zsb[:, t * H:(t + 1) * H], in0=pz[:],
                    scalar1=dis2col_sb[:, t:t + 1], scalar2=None,
                    op0=mybir.AluOpType.mult)
                if g == 3 or t == T - 1:
                    t0 = t - g
                    bo = bounces[layer][:].rearrange(
                        "(t p) d -> p t d", p=128)
                    nc.sync.dma_start(
                        bo[:, t0:t + 1, 0:H], zst[:, 0:g + 1, :])
                    # fill the 256B-row padding too (keeps zfull finite)
                    nc.sync.dma_start(
                        bo[:, t0:t + 1, H:2 * H], zst[:, 0:g + 1, :])

            # ---------------- layer 0: z0 = dis * (x @ W1) ----------------
            zsb = zsbp.tile([128, T * H], BF16, tag="zsb")
            zst = None
            for t in range(T):
                if t % 4 == 0:
                    zst = zst_p.tile([128, 4, H], BF16, tag="zst")
                pz = ps_z.tile([128, H], F32, space="PSUM", tag="pz")
                nc.tensor.matmul(out=pz[:], lhsT=xT_sb[:, t * 128:(t + 1) * 128],
                                 rhs=W1_sb[:], start=True, stop=True)
                write_z(0, pz, t, zst, zsb)
            nc.gpsimd.collective_compute(
                "AllGather", mybir.AluOpType.bypass,
                replica_groups=[list(range(M))],
                ins=[bounces[0][:]], outs=[zfulls[0][:]])

            # ---------------- layers 1..5 ----------------
            qn = 0
            reg_cache = {}

            def nidx_reg(n):
                if n not in reg_cache:
                    reg_cache[n] = nc.gpsimd.to_reg(n)
                return reg_cache[n]

            for layer in range(1, 6):
                zsb_prev = zsb
                if layer < 5:
                    zsb = zsbp.tile([128, T * H], BF16, tag="zsb")
                else:
                    ppool = ps_pool.tile([H, 128], F32, space="PSUM", tag="ppool")
                zf = zfulls[layer - 1]
                for R, tiles in enumerate(ranges):
                    nbR = range_nb[R]
                    oR = range_off[R]
                    gR = gp.tile([128, NBR, 128], BF16, tag="gR")
                    for r in range(NCH):
                        nb = nb_call[(R, r)]
                        if nb == 0:
                            continue
                        o = call_off[(R, r)] - oR
                        rows = min(CH, NSTOT - r * CH)
                        nc.gpsimd.dma_gather(
                            gR[:, o:o + nb, :],
                            zf[r * CH:r * CH + rows, :],
                            idx_sb[:, call_off[(R, r)] * 8:
                                   (call_off[(R, r)] + nb) * 8],
                            nb * 128, nidx_reg(nb * 128), 128,
                            single_packet=False, queue_num=qn % 4)
                        qn += 1
                    # one-hot masks for the whole range; dis[dst] onto g
                    mR = mp.tile([128, NBR, 128], BF16, tag="mR")
                    nc.vector.tensor_tensor(
                        out=mR[:, 0:nbR, :],
                        in0=iota_sb[:, None, :].to_broadcast([128, nbR, 128]),
                        in1=ldst_sb[:, oR:oR + nbR, None].to_broadcast(
                            [128, nbR, 128]),
                        op=mybir.AluOpType.is_equal)
                    nc.vector.tensor_tensor(
                        out=gR[:, 0:nbR, 0:H],
                        in0=gR[:, 0:nbR, 0:H],
                        in1=disd_sb[:, oR:oR + nbR, None].to_broadcast(
                            [128, nbR, H]),
                        op=mybir.AluOpType.mult)
                    for t in tiles:
                        if layer < 5:
                            pagg = ps_agg.tile([H, 128], F32, space="PSUM",
                                               tag="pagg")
                            first = True
                            for r in range(NCH):
                                o = blk_off[t, r] - oR
                                for b in range(B_tr[t, r]):
                                    nc.tensor.matmul(
                                        out=pagg[:],
                                        lhsT=gR[:, o + b, 0:H],
                                        rhs=mR[:, o + b, :],
                                        start=first, stop=False)
                                    first = False
                            nc.tensor.matmul(
                                out=pagg[:],
                                lhsT=zsb_prev[:, t * H:(t + 1) * H],
                                rhs=ident_sb[:],
                                start=first, stop=True)
                            hT = hTp.tile([H, 128], BF16, tag="hT")
                            nc.scalar.activation(
                                hT[:], pagg[:],
                                mybir.ActivationFunctionType.Relu,
                                bias=b14_sb[:, layer - 1:layer])
                            if t % 4 == 0:
                                zst = zst_p.tile([128, 4, H], BF16, tag="zst")
                            pz = ps_z.tile([128, H], F32, space="PSUM", tag="pz")
                            nc.tensor.matmul(
                                out=pz[:], lhsT=hT[:],
                                rhs=Wh2_sb[:, (layer - 1) * H:layer * H],
                                start=True, stop=True)
                            write_z(layer, pz, t, zst, zsb)
                        else:
                            p5 = ps_agg.tile([128, H], F32, space="PSUM",
                                             tag="pagg")
                            first = True
                            for r in range(NCH):
                                o = blk_off[t, r] - oR
                                for b in range(B_tr[t, r]):
                                    nc.tensor.matmul(
                                        out=p5[:],
                                        lhsT=mR[:, o + b, :],
                                        rhs=gR[:, o + b, 0:H],
                                        start=first, stop=False)
                                    first = False
                            nc.tensor.matmul(
                                out=p5[:], lhsT=ident_sb[:],
                                rhs=zsb_prev[:, t * H:(t + 1) * H],
                                start=first, stop=False)
                            nc.tensor.matmul(
                                out=p5[:], lhsT=ones1_sb[:], rhs=b5row_sb[:],
                                start=False, stop=True)
                            h5 = h5p.tile([128, H], BF16, tag="h5")
                            nc.scalar.activation(
                                h5[:], p5[:], mybir.ActivationFunctionType.Relu)
                            pt = ptp.tile([128, 128], BF16, tag="pt")
                            nc.vector.tensor_tensor(
                                out=pt[:], in0=iota_sb[:],
                                in1=gcol_sb[:, t:t + 1].to_broadcast([128, 128]),
                                op=mybir.AluOpType.is_equal)
                            nc.tensor.matmul(
                                out=ppool[:], lhsT=h5[:], rhs=pt[:],
                                start=(t == 0), stop=(t == T - 1))
                if layer < 5:
                    nc.gpsimd.collective_compute(
                        "AllGather", mybir.AluOpType.bypass,
                        replica_groups=[list(range(M))],
                        ins=[bounces[layer][:]], outs=[zfulls[layer][:]])

            # ---------------- mean-pool, layernorm, head ----------------
            poolT = ep.tile([H, 128], F32, tag="poolT")
            nc.scalar.copy(poolT[:], ppool[:])
            ptr = ps_misc.tile([128, H], F32, space="PSUM", tag="ptr")
            nc.tensor.transpose(out=ptr[:], in_=poolT[:], identity=identH[:])
            pooled = ep.tile([128, H], F32, tag="pooled")
            nc.vector.tensor_scalar(out=pooled[:], in0=ptr[:],
                                    scalar1=invcnt_sb[:, 0:1],
                                    scalar2=None, op0=mybir.AluOpType.mult)
            mu = ep.tile([128, 1], F32, tag="mu")
            nc.vector.tensor_reduce(out=mu[:], in_=pooled[:],
                                    axis=mybir.AxisListType.X,
                                    op=mybir.AluOpType.add)
            nc.vector.tensor_scalar(out=mu[:], in0=mu[:], scalar1=1.0 / H,
                                    scalar2=None, op0=mybir.AluOpType.mult)
            xc = ep.tile([128, H], F32, tag="xc")
            nc.vector.tensor_scalar(out=xc[:], in0=pooled[:],
                                    scalar1=mu[:, 0:1],
                                    scalar2=None, op0=mybir.AluOpType.subtract)
            sq = ep.tile([128, H], F32, tag="sq")
            nc.scalar.activation(sq[:], xc[:],
                                 mybir.ActivationFunctionType.Square)
            var = ep.tile([128, 1], F32, tag="var")
            nc.vector.tensor_reduce(out=var[:], in_=sq[:],
                                    axis=mybir.AxisListType.X,
                                    op=mybir.AluOpType.add)
            nc.vector.tensor_scalar(out=var[:], in0=var[:], scalar1=1.0 / H,
                                    scalar2=None, op0=mybir.AluOpType.mult)
            eps_col = ep.tile([128, 1], F32, tag="eps")
            nc.gpsimd.memset(eps_col[:], 1e-5)
            std = ep.tile([128, 1], F32, tag="std")
            nc.scalar.activation(std[:], var[:],
                                 mybir.ActivationFunctionType.Sqrt,
                                 bias=eps_col[:, 0:1])
            rstd = ep.tile([128, 1], F32, tag="rstd")
            nc.vector.reciprocal(rstd[:], std[:])
            ln = ep.tile([128, H], F32, tag="ln")
            nc.vector.tensor_scalar(out=ln[:], in0=xc[:], scalar1=rstd[:, 0:1],
                                    scalar2=None, op0=mybir.AluOpType.mult)
            y = ep.tile([128, H], F32, tag="y")
            nc.vector.tensor_tensor(out=y[:], in0=ln[:], in1=woutrep_sb[:],
                                    op=mybir.AluOpType.mult)
            yr = ep.tile([128, 1], F32, tag="yr")
            nc.vector.tensor_reduce(out=yr[:], in_=y[:],
                                    axis=mybir.AxisListType.X,
                                    op=mybir.AluOpType.add)
            nc.vector.tensor_scalar(out=yr[:], in0=yr[:], scalar1=bout,
                                    scalar2=None, op0=mybir.AluOpType.add)
            nc.sync.dma_start(out_d[:], yr[:])

    mybir.codegen_inst_isa_subclasses(nc)
    return nc


# ---------------------------------------------------------------------------
# Entry point
# ---------------------------------------------------------------------------

def kernel(x, edge_index, batch, W1, b1, Wh, bh, Wout, bout):
    from concourse.bass_utils import run_bass_kernel_spmd

    x = np.asarray(x, np.float32)
    in_maps, geom = preprocess(x, edge_index, batch)
    wmaps, wmeta = make_weight_inputs(W1, b1, Wh, bh, Wout, bout)
    for im in in_maps:
        im.update(wmaps)
    nc = build_nc(geom, wmeta["bout"])

    import time
    last_err = None
    for attempt in range(3):
        try:
            res = run_bass_kernel_spmd(nc, in_maps, core_ids=list(range(M)))
            break
        except Exception as e:  # transient terminal hiccups / device recovery
            last_err = e
            time.sleep(30 * (attempt + 1))
    else:
        raise last_err

    out = np.concatenate([res.results[c]["out"][:GPC] for c in range(M)], axis=0)
    return np.ascontiguousarray(out, np.float32)
